# revision 1
# baseline (speedup 1.0000x reference)
"""Binary 3-layer CNN (sign activations + sign weights) on 8 NeuronCores.

Strategy: pure data parallel — 64 images -> 8 cores x 8 images.
Per core: 2 batches of 4 images; SBUF partition layout [128 = (4 img, 32 ch), pix].
Each 3x3 conv = 9 PSUM-accumulating matmuls with block-diagonal stationary
weights (4 identical 32x32 blocks) and free-dim-shifted rhs APs (dy*Wpad+dx),
so there is no im2col data movement. All matmul operands are exactly +-1/0 ->
bf16 with fp32 PSUM accumulation is numerically exact. sign() is applied by
ScalarE during PSUM->SBUF eviction. Layers staged through DRAM scratch in a
zero-padded layout (pad rows AND pad cols pre-zeroed in DRAM once) so conv
padding is baked in and SBUF tiles are single-producer.
"""

import numpy as np
import ml_dtypes

import concourse.bass as bass
import concourse.mybir as mybir
import concourse.tile as tile
from concourse import bacc
from concourse.bass_utils import run_bass_kernel_spmd

BF16 = mybir.dt.bfloat16
F32 = mybir.dt.float32
AF = mybir.ActivationFunctionType

N_CORES = 8
IMG_PER_CORE = 8
B = 4          # images per partition-batch
H = W = 256
WP = 258       # padded width (1 col pad each side)
HP = 258       # padded height
R = 64         # strip rows (stages A/B)
RC = 32        # strip rows (stage C)
NB = IMG_PER_CORE // B  # batches per core


def _conv_strip(nc, pspool, lhsT_taps, hin, dst_evict, rows):
    """rows output rows; hin is [*, rows+2, WP] (padded); evict 4 psum tiles."""
    mparts = lhsT_taps[0].shape[-1]
    for r0 in range(0, rows, 8):  # groups of 4 psum tiles (2 rows each)
        pss = [pspool.tile([mparts, 2, 256], F32, tag=f"ps{p}", name=f"ps{p}")
               for p in range(4)]
        for tap in range(9):
            dy, dx = tap // 3, tap % 3
            for p in range(4):
                r = r0 + 2 * p
                nc.tensor.matmul(
                    pss[p][:, :, :],
                    lhsT_taps[tap],
                    hin[:, r + dy:r + dy + 2, dx:dx + 256],
                    start=(tap == 0), stop=(tap == 8))
        dst_evict(pss, r0)


def _build_program(stages=('0','A','B','C')):
    nc = bacc.Bacc("TRN2", target_bir_lowering=False, debug=False)

    x_in = nc.dram_tensor("x", [IMG_PER_CORE, H, W], F32, kind="ExternalInput")
    s0_in = nc.dram_tensor("s0", [12, 3, 128], BF16, kind="ExternalInput")
    s1_in = nc.dram_tensor("s1", [128, 9, 128], BF16, kind="ExternalInput")
    s2_in = nc.dram_tensor("s2", [128, 9, B], BF16, kind="ExternalInput")
    out_d = nc.dram_tensor("out", [IMG_PER_CORE, H, W], F32, kind="ExternalOutput")

    xs_d = nc.dram_tensor("xs", [IMG_PER_CORE, HP, WP], BF16)
    h0_d = nc.dram_tensor("h0", [NB, 128, HP, WP], BF16)
    h1_d = nc.dram_tensor("h1", [NB, 128, HP, WP], BF16)

    with tile.TileContext(nc) as tc:
        with (
            tc.tile_pool(name="const", bufs=1) as cpool,
            tc.tile_pool(name="xprep", bufs=4) as xpool,
            tc.tile_pool(name="ain", bufs=2) as apool,
            tc.tile_pool(name="aout", bufs=2) as opool,
            tc.tile_pool(name="cout", bufs=1) as cpool2,
            tc.tile_pool(name="psum", bufs=2, space="PSUM") as pspool,
        ):
            # --- constants: stationary weights + a zero tile ---
            # s0 replicated into 4 row groups (base partitions 0/32/64/96)
            # so 4 psum tiles' conv0 matmuls run concurrently via row tiling
            s0t = cpool.tile([108, 3, 128], BF16, tag="s0")
            for p in range(4):
                nc.sync.dma_start(out=s0t[32 * p:32 * p + 12, :, :],
                                  in_=s0_in[:, :, :])
            s1t = cpool.tile([128, 9, 128], BF16, tag="s1")
            nc.sync.dma_start(out=s1t[:, :, :], in_=s1_in[:, :, :])
            s2t = cpool.tile([128, 9, B], BF16, tag="s2")
            nc.sync.dma_start(out=s2t[:, :, :], in_=s2_in[:, :, :])
            zt = cpool.tile([128, WP], BF16, tag="zt")
            nc.gpsimd.memset(zt[:, :], 0.0)

            # --- pre-zero DRAM pad rows (contiguous); col pads are baked
            # into the SBUF tiles below ---
            for img in range(IMG_PER_CORE):
                nc.scalar.dma_start(out=xs_d[img, 0:1, :], in_=zt[0:1, :])
                nc.scalar.dma_start(out=xs_d[img, HP - 1:HP, :], in_=zt[0:1, :])
            for b in range(NB):
                for hd in (h0_d, h1_d):
                    nc.scalar.dma_start(out=hd[b, :, 0, :], in_=zt[:, :])
                    nc.scalar.dma_start(out=hd[b, :, HP - 1, :], in_=zt[:, :])

            # --- stage 0: sign(x) -> padded bf16 planes in DRAM ---
            for img in range(IMG_PER_CORE if '0' in stages else 0):
                for rb in range(H // 128):
                    xf = xpool.tile([128, W], F32, tag="xf")
                    nc.sync.dma_start(
                        out=xf[:, :], in_=x_in[img, rb * 128:(rb + 1) * 128, :])
                    xp = xpool.tile([128, WP], BF16, tag="xp")
                    nc.scalar.activation(xp[:, 1:W + 1], xf[:, :], AF.Sign)
                    nc.vector.memset(xp[:, 0:1], 0.0)
                    nc.vector.memset(xp[:, WP - 1:WP], 0.0)
                    nc.scalar.dma_start(
                        out=xs_d[img, rb * 128 + 1:(rb + 1) * 128 + 1, :],
                        in_=xp[:, :])

            for b in range(NB):
                # ---- stage A: conv0 (1 -> 32ch), dy-in-K: K=12, M=128,
                # 4x row tiling: input replicated to partition groups
                # 0/32/64/96; the 4 psum tiles' matmuls occupy distinct
                # 32-row strips of the PE array and run concurrently ----
                for s in range(H // R if 'A' in stages else 0):
                    xt = apool.tile([108, R, WP], BF16, tag="lin")
                    for p in range(4):
                        for dy in range(3):
                            nc.sync.dma_start(
                                out=xt[32 * p + dy * B:32 * p + (dy + 1) * B,
                                       :, :],
                                in_=xs_d[b * B:(b + 1) * B,
                                         s * R + dy:s * R + dy + R, :])
                    ht = opool.tile([128, R, WP], BF16, tag="a_out")
                    nc.vector.memset(ht[:, :, 0:1], 0.0)
                    nc.vector.memset(ht[:, :, WP - 1:WP], 0.0)
                    for r0 in range(0, R, 8):
                        pss = [pspool.tile([128, 2, 256], F32,
                                           tag=f"ps{p}", name=f"ps{p}")
                               for p in range(4)]
                        for dx in range(3):
                            for p in range(4):
                                r = r0 + 2 * p
                                nc.tensor.matmul(
                                    pss[p][:, :, :],
                                    s0t[32 * p:32 * p + 12, dx, :],
                                    xt[32 * p:32 * p + 12, r:r + 2,
                                       dx:dx + 256],
                                    start=(dx == 0), stop=(dx == 2),
                                    tile_position=(32 * p, 0))
                        for p in range(4):
                            r = r0 + 2 * p
                            nc.scalar.activation(
                                ht[:, r:r + 2, 1:W + 1], pss[p][:, :, :], AF.Sign)
                    nc.scalar.dma_start(
                        out=h0_d[b, :, s * R + 1:s * R + R + 1, :],
                        in_=ht[:, :, :])

                # ---- stage B: conv1 (32 -> 32ch), K=128, M=128 ----
                for s in range(H // R if 'B' in stages else 0):
                    hin = apool.tile([128, R + 2, WP], BF16, tag="lin")
                    nc.sync.dma_start(
                        out=hin[:, :, :], in_=h0_d[b, :, s * R:s * R + R + 2, :])
                    ht = opool.tile([128, R, WP], BF16, tag="a_out")
                    nc.vector.memset(ht[:, :, 0:1], 0.0)
                    nc.vector.memset(ht[:, :, WP - 1:WP], 0.0)

                    def evict_b(pss, r0, ht=ht):
                        for p in range(4):
                            r = r0 + 2 * p
                            nc.scalar.activation(
                                ht[:, r:r + 2, 1:W + 1], pss[p][:, :, :], AF.Sign)

                    _conv_strip(nc, pspool,
                                [s1t[:, t, :] for t in range(9)], hin, evict_b, R)
                    nc.scalar.dma_start(
                        out=h1_d[b, :, s * R + 1:s * R + R + 1, :],
                        in_=ht[:, :, :])

                # ---- stage C: conv2 (32 -> 1ch), K=128, M=4, 4x col-tiling ----
                # 4 psum row-pairs go to col groups 0/32/64/96 of the SAME
                # psum tile; the 4 matmuls per tap run concurrently on
                # distinct 32-col strips of the PE array.
                for s in range(H // RC if 'C' in stages else 0):
                    hin = apool.tile([128, RC + 2, WP], BF16, tag="lin")
                    nc.sync.dma_start(
                        out=hin[:, :, :], in_=h1_d[b, :, s * RC:s * RC + RC + 2, :])
                    ot = cpool2.tile([B, RC, W], F32, tag="c_out")
                    for r0 in range(0, RC, 8):
                        ps = pspool.tile([128, 2, 256], F32, tag="ps0", name="psc")
                        for tap in range(9):
                            dy, dx = tap // 3, tap % 3
                            for p in range(4):
                                r = r0 + 2 * p
                                nc.tensor.matmul(
                                    ps[32 * p:32 * p + B, :, :],
                                    s2t[:, tap, :],
                                    hin[:, r + dy:r + dy + 2, dx:dx + 256],
                                    start=(tap == 0), stop=(tap == 8),
                                    tile_position=(0, 32 * p))
                        for p in range(4):
                            r = r0 + 2 * p
                            nc.vector.tensor_copy(
                                ot[:, r:r + 2, :], ps[32 * p:32 * p + B, :, :])
                    for g in range(B):
                        nc.scalar.dma_start(
                            out=out_d[b * B + g, s * RC:s * RC + RC, :],
                            in_=ot[g:g + 1, :, :])
    nc.compile()
    return nc


def _host_weights(w0, w1, w2):
    """Build bf16 block-diag stationary matrices. tap index = dy*3+dx."""
    sg = lambda w: np.sign(w).astype(ml_dtypes.bfloat16)
    w0s, w1s, w2s = sg(w0), sg(w1), sg(w2)   # [32,1,3,3],[32,32,3,3],[1,32,3,3]
    s0 = np.zeros((12, 3, 128), ml_dtypes.bfloat16)
    s1 = np.zeros((128, 9, 128), ml_dtypes.bfloat16)
    s2 = np.zeros((128, 9, B), ml_dtypes.bfloat16)
    for g in range(B):
        for dy in range(3):
            for dx in range(3):
                t = dy * 3 + dx
                # lhsT[k, m]: out[m] += sum_k lhsT[k,m]*rhs[k]
                # s0 [dy*4+g, dx, g*32+co]
                s0[dy * B + g, dx, g * 32:(g + 1) * 32] = w0s[:, 0, dy, dx]
                s1[g * 32:(g + 1) * 32, t, g * 32:(g + 1) * 32] = \
                    w1s[:, :, dy, dx].T  # [ci, co]
                s2[g * 32:(g + 1) * 32, t, g] = w2s[0, :, dy, dx]
    return s0, s1, s2


_NC_CACHE = {}


def kernel(x, w0, w1, w2):
    if "nc" not in _NC_CACHE:
        _NC_CACHE["nc"] = _build_program()
    nc = _NC_CACHE["nc"]
    s0, s1, s2 = _host_weights(np.asarray(w0), np.asarray(w1), np.asarray(w2))
    x = np.asarray(x, np.float32).reshape(64, H, W)
    in_maps = [
        {"x": np.ascontiguousarray(x[i * IMG_PER_CORE:(i + 1) * IMG_PER_CORE]),
         "s0": s0, "s1": s1, "s2": s2}
        for i in range(N_CORES)
    ]
    res = run_bass_kernel_spmd(nc, in_maps, list(range(N_CORES)))
    out = np.stack([np.asarray(res.results[i]["out"], np.float32)
                    for i in range(N_CORES)])
    return out.reshape(64, 1, H, W)



# revision 7
# speedup vs baseline: 15.1188x; 15.1188x over previous
"""Binary 3-layer CNN (sign activations + sign weights) on 8 NeuronCores.

Strategy: pure data parallel — 64 images -> 8 cores x 8 images.
Per core: 2 batches of 4 images; SBUF partition layout [128 = (4 img, 32 ch), pix].
Each 3x3 conv is computed as PSUM-accumulating fp8 DoubleRow matmuls (K=256
contracted per pass at 2x the bf16 rate). The two DoubleRow k-tiles hold:
  - stage A/B: a pair of dy-taps (dy0&dy2, or dy1&zero) — row-shifted windows
    whose APs are disjoint with strictly descending strides (the PE ifmap AP
    generator rejects overlapping/ascending k-tile strides);
  - stage C: the two image-batches (plane-strided, disjoint).
All matmul operands are exactly +-1/0 -> fp8e4 with fp32 PSUM accumulation is
numerically exact. sign() is applied during PSUM->SBUF eviction (sums are
integer-valued, so clip(x,-1,1) == sign(x)); intermediates are staged through
DRAM in a zero-padded fp8 layout (padding baked in, tiles single-producer).
The final conv (32->1ch) emits f16 (exact for |sum|<=288).

Dispatch: the jitted shard_map callable is built ONCE and cached, so warm
kernel() calls skip re-lowering/re-compile and only pay transfer + execute.
"""

import numpy as np
import ml_dtypes

import jax
import concourse.bass as bass
import concourse.mybir as mybir
import concourse.tile as tile
from concourse import bacc
from concourse.bass_utils import run_bass_kernel_spmd  # noqa: F401 (debug path)
from concourse.bass2jax import (
    _bass_exec_p, partition_id_tensor, install_neuronx_cc_hook)
from jax.experimental.shard_map import shard_map
from jax.sharding import Mesh, PartitionSpec

F32 = mybir.dt.float32
F16 = mybir.dt.float16
FP8 = mybir.dt.float8e4
E4 = ml_dtypes.float8_e4m3
AF = mybir.ActivationFunctionType
DR = mybir.MatmulPerfMode.DoubleRow
ALU = mybir.AluOpType

N_CORES = 8
IMG_PER_CORE = 8
B = 4          # images per partition-batch
H = W = 256
WP = 258       # padded width (1 col pad each side)
HPX = 260      # xs rows: pad row 0, data 1..256, zero rows 257..259
HPH = 260      # h0/h1 rows: pad row 0, data 1..256, zero rows 257..259
R = 64         # strip rows (stages A/B)
RC = 16        # strip rows (stage C)
NB = IMG_PER_CORE // B  # batches per core
TW = 256       # stage-A tile row width (dx pre-shifted)


def _build_program(stages=('0', 'A', 'B', 'C')):
    nc = bacc.Bacc("TRN2", target_bir_lowering=False, debug=False)

    x_in = nc.dram_tensor("x", [IMG_PER_CORE, H, W], F32, kind="ExternalInput")
    s0_in = nc.dram_tensor("s0", [12, 2, 2, 128], FP8, kind="ExternalInput")
    s1_in = nc.dram_tensor("s1", [128, 6, 2, 128], FP8, kind="ExternalInput")
    # stage-C M padded 8 -> 16: DoubleRow ldweights requires the k-tile
    # stride to be a multiple of 16 bytes (s3_lw dual-fp8 restriction).
    s2_in = nc.dram_tensor("s2", [128, 9, 2, 16], FP8, kind="ExternalInput")
    out_d = nc.dram_tensor("out", [IMG_PER_CORE, H, W], F16, kind="ExternalOutput")

    xs_d = nc.dram_tensor("xs", [IMG_PER_CORE, HPX, WP], FP8)
    h0_d = nc.dram_tensor("h0", [NB, 128, HPH, WP], FP8)
    h1_d = nc.dram_tensor("h1", [NB, 128, HPH, WP], FP8)

    with tile.TileContext(nc) as tc:
        with (
            tc.tile_pool(name="const", bufs=1) as cpool,
            tc.tile_pool(name="xprep", bufs=3) as xpool,
            tc.tile_pool(name="ain", bufs=2) as apool,
            tc.tile_pool(name="aout", bufs=2) as aopool,
            tc.tile_pool(name="bin", bufs=2) as bpool,
            tc.tile_pool(name="bout", bufs=2) as bopool,
            tc.tile_pool(name="cin", bufs=2) as cpool2,
            tc.tile_pool(name="cout", bufs=2) as copool,
            tc.tile_pool(name="psum", bufs=2, space="PSUM") as pspool,
        ):
            # --- constants: stationary weights + a zero tile ---
            s0t = cpool.tile([12, 2, 2, 128], FP8, tag="s0")
            nc.sync.dma_start(out=s0t[:, :, :, :], in_=s0_in[:, :, :, :])
            s1t = cpool.tile([128, 6, 2, 128], FP8, tag="s1")
            nc.sync.dma_start(out=s1t[:, :, :, :], in_=s1_in[:, :, :, :])
            s2t = cpool.tile([128, 9, 2, 16], FP8, tag="s2")
            nc.sync.dma_start(out=s2t[:, :, :, :], in_=s2_in[:, :, :, :])
            zt = cpool.tile([128, 3 * WP], FP8, tag="zt")
            nc.gpsimd.memset(zt[:, :], 0.0)

            # --- pre-zero DRAM pad rows; col pads are baked into tiles ---
            for img in range(IMG_PER_CORE):
                nc.scalar.dma_start(out=xs_d[img, 0:1, :], in_=zt[0:1, 0:WP])
                nc.scalar.dma_start(
                    out=xs_d[img, H + 1:HPX, :],
                    in_=zt[0:1, 0:(HPX - H - 1) * WP])
            for b in range(NB):
                for hd in (h0_d, h1_d):
                    nc.scalar.dma_start(out=hd[b, :, 0, :], in_=zt[:, 0:WP])
                    nc.scalar.dma_start(
                        out=hd[b, :, H + 1:HPH, :],
                        in_=zt[:, 0:(HPH - H - 1) * WP])

            # --- stage 0: sign(x) -> padded fp8 planes in DRAM ---
            for img in range(IMG_PER_CORE if '0' in stages else 0):
                for rb in range(H // 128):
                    xf = xpool.tile([128, W], F32, tag="xf")
                    nc.sync.dma_start(
                        out=xf[:, :], in_=x_in[img, rb * 128:(rb + 1) * 128, :])
                    xp = xpool.tile([128, WP], FP8, tag="xp")
                    nc.scalar.activation(xp[:, 1:W + 1], xf[:, :], AF.Sign)
                    nc.vector.memset(xp[:, 0:1], 0.0)
                    nc.vector.memset(xp[:, WP - 1:WP], 0.0)
                    nc.scalar.dma_start(
                        out=xs_d[img, rb * 128 + 1:(rb + 1) * 128 + 1, :],
                        in_=xp[:, :])

            for b in range(NB):
                # ---- stage A: conv0 (1 -> 32ch). Partitions = (dx, img):
                # tile row i, col c = xs[img, sR+i, c+dx]. DoubleRow k-tiles
                # = dy pairs: j0=(dy0, dy2), j1=(dy1, zero); kt stride 2*TW.
                for s in range(H // R if 'A' in stages else 0):
                    xt = apool.tile([12, R + 4, TW], FP8, tag="xt")
                    for dx in range(3):
                        nc.sync.dma_start(
                            out=xt[4 * dx:4 * dx + 4, :, :],
                            in_=xs_d[b * B:(b + 1) * B,
                                     s * R:s * R + R + 4, dx:dx + TW])
                    ht = aopool.tile([128, R, WP], FP8, tag="a_out")
                    nc.vector.memset(ht[:, :, 0:1], 0.0)
                    nc.vector.memset(ht[:, :, WP - 1:WP], 0.0)
                    xbase = xt[:, 0:1, 0:1]
                    xpd = list(xbase.ap[0])
                    for r0 in range(0, R, 8):
                        pss = [pspool.tile([128, 2, 256], F32,
                                           tag=f"ps{p}", name=f"ps{p}")
                               for p in range(4)]
                        for j in range(2):
                            for p in range(4):
                                r = r0 + 2 * p
                                rhs = bass.AP(
                                    xbase.tensor,
                                    xbase.offset + (r + j) * TW,
                                    [xpd, [2 * TW, 2], [TW, 2], [1, 256]])
                                nc.tensor.matmul(
                                    pss[p][:, :, :], s0t[:, j, :, :], rhs,
                                    start=(j == 0), stop=(j == 1),
                                    perf_mode=DR)
                        for p in range(4):
                            r = r0 + 2 * p
                            nc.vector.tensor_scalar(
                                ht[:, r:r + 2, 1:W + 1], pss[p][:, :, :],
                                -1.0, 1.0, op0=ALU.max, op1=ALU.min)
                    nc.scalar.dma_start(
                        out=h0_d[b, :, s * R + 1:s * R + R + 1, :],
                        in_=ht[:, :, :])

                # ---- stage B: conv1 (32 -> 32ch), K=128 block-diag.
                # DoubleRow k-tiles = dy pairs per dx: j in 0..5:
                #   j<3:  (dy0,dx=j) & (dy2,dx=j)     kt stride 2*WP
                #   j>=3: (dy1,dx=j-3) & zero-weights kt stride 2*WP
                for s in range(H // R if 'B' in stages else 0):
                    hin = bpool.tile([128, R + 4, WP], FP8, tag="b_in")
                    nc.sync.dma_start(
                        out=hin[:, :, :],
                        in_=h0_d[b, :, s * R:s * R + R + 4, :])
                    ht = bopool.tile([128, R, WP], FP8, tag="b_out")
                    nc.vector.memset(ht[:, :, 0:1], 0.0)
                    nc.vector.memset(ht[:, :, WP - 1:WP], 0.0)
                    hbase = hin[:, 0:1, 0:1]
                    hpd = list(hbase.ap[0])
                    for r0 in range(0, R, 8):
                        pss = [pspool.tile([128, 2, 256], F32,
                                           tag=f"ps{p}", name=f"ps{p}")
                               for p in range(4)]
                        for j in range(6):
                            dy0 = 0 if j < 3 else 1
                            dx = j % 3
                            for p in range(4):
                                r = r0 + 2 * p
                                rhs = bass.AP(
                                    hbase.tensor,
                                    hbase.offset + (r + dy0) * WP + dx,
                                    [hpd, [2 * WP, 2], [WP, 2], [1, 256]])
                                nc.tensor.matmul(
                                    pss[p][:, :, :], s1t[:, j, :, :], rhs,
                                    start=(j == 0), stop=(j == 5),
                                    perf_mode=DR)
                        for p in range(4):
                            r = r0 + 2 * p
                            nc.vector.tensor_scalar(
                                ht[:, r:r + 2, 1:W + 1], pss[p][:, :, :],
                                -1.0, 1.0, op0=ALU.max, op1=ALU.min)
                    nc.scalar.dma_start(
                        out=h1_d[b, :, s * R + 1:s * R + R + 1, :],
                        in_=ht[:, :, :])

            # ---- stage C: conv2 (32 -> 1ch), both batches at once:
            # DoubleRow k-tiles = the 2 image-batches (K=256), M=8 imgs ----
            for s in range(H // RC if 'C' in stages else 0):
                hin = cpool2.tile([128, 2, RC + 2, WP], FP8, tag="c_in")
                for b in range(NB):
                    nc.sync.dma_start(
                        out=hin[:, b, :, :],
                        in_=h1_d[b, :, s * RC:s * RC + RC + 2, :])
                ot = copool.tile([8, RC, W], F16, tag="c_out")
                plane = (RC + 2) * WP
                cbase = hin[:, 0:1, 0:1, 0:1]
                cpd = list(cbase.ap[0])
                for r0 in range(0, RC, 8):
                    pss = [pspool.tile([128, 2, 256], F32,
                                       tag=f"ps{p}", name=f"ps{p}")
                           for p in range(4)]
                    for t in range(9):
                        dy, dx = t // 3, t % 3
                        for p in range(4):
                            r = r0 + 2 * p
                            rhs = bass.AP(
                                cbase.tensor,
                                cbase.offset + (r + dy) * WP + dx,
                                [cpd, [plane, 2], [WP, 2], [1, 256]])
                            nc.tensor.matmul(
                                pss[p][0:16, :, :], s2t[:, t, :, :], rhs,
                                start=(t == 0), stop=(t == 8),
                                perf_mode=DR)
                    for p in range(4):
                        r = r0 + 2 * p
                        nc.scalar.activation(
                            ot[:, r:r + 2, :], pss[p][0:8, :, :], AF.Copy)
                nc.scalar.dma_start(
                    out=out_d[0:IMG_PER_CORE, s * RC:(s + 1) * RC, :],
                    in_=ot[:, :, :])
    nc.compile()
    return nc


def _host_weights(w0, w1, w2):
    """Pack sign(w) into fp8 DoubleRow stationary layouts."""
    sg = lambda w: np.sign(np.asarray(w, np.float32))
    w0s, w1s, w2s = sg(w0), sg(w1), sg(w2)  # [32,1,3,3],[32,32,3,3],[1,32,3,3]
    s0 = np.zeros((12, 2, 2, 128), np.float32)
    s1 = np.zeros((128, 6, 2, 128), np.float32)
    s2 = np.zeros((128, 9, 2, 16), np.float32)
    for g in range(B):
        ms = slice(g * 32, (g + 1) * 32)
        for dx in range(3):
            p = dx * 4 + g
            # lhsT[k, kt, m]: out[m] += sum_k sum_kt lhsT[k,kt,m]*rhs[k,kt]
            s0[p, 0, 0, ms] = w0s[:, 0, 0, dx]
            s0[p, 0, 1, ms] = w0s[:, 0, 2, dx]
            s0[p, 1, 0, ms] = w0s[:, 0, 1, dx]
        for j in range(6):
            dx = j % 3
            if j < 3:
                s1[ms, j, 0, ms] = w1s[:, :, 0, dx].T  # [ci, co]
                s1[ms, j, 1, ms] = w1s[:, :, 2, dx].T
            else:
                s1[ms, j, 0, ms] = w1s[:, :, 1, dx].T
        for t in range(9):
            dy, dx = t // 3, t % 3
            for kt in range(NB):
                s2[ms, t, kt, kt * 4 + g] = w2s[0, :, dy, dx]
    return s0.astype(E4), s1.astype(E4), s2.astype(E4)


def _make_runner(nc):
    """Build the jitted shard_map dispatcher ONCE (cached across calls)."""
    install_neuronx_cc_hook()
    partition_name = nc.partition_id_tensor.name if nc.partition_id_tensor else None
    in_names, out_names, out_avals, zero_shapes = [], [], [], []
    for alloc in nc.m.functions[0].allocations:
        if not isinstance(alloc, mybir.MemoryLocationSet):
            continue
        name = alloc.memorylocations[0].name
        if alloc.kind == "ExternalInput":
            if name != partition_name:
                in_names.append(name)
        elif alloc.kind == "ExternalOutput":
            out_names.append(name)
            shape = tuple(alloc.tensor_shape)
            dtype = mybir.dt.np(alloc.dtype)
            out_avals.append(jax.core.ShapedArray(shape, dtype))
            zero_shapes.append((shape, dtype))
    n_params = len(in_names)
    all_in_names = list(in_names) + list(out_names)
    if partition_name is not None:
        all_in_names.append(partition_name)

    def _body(*args):
        operands = list(args)
        if partition_name is not None:
            operands.append(partition_id_tensor())
        outs = _bass_exec_p.bind(
            *operands,
            out_avals=tuple(out_avals),
            in_names=tuple(all_in_names),
            out_names=tuple(out_names),
            lowering_input_output_aliases=(),
            sim_require_finite=True,
            sim_require_nnan=True,
            nc=nc,
        )
        return tuple(outs)

    devices = jax.devices()[:N_CORES]
    mesh = Mesh(np.asarray(devices), ("core",))
    n_outs = len(out_names)
    in_specs = (PartitionSpec("core"),) * (n_params + n_outs)
    out_specs = (PartitionSpec("core"),) * n_outs
    sharded = jax.jit(
        shard_map(_body, mesh=mesh, in_specs=in_specs, out_specs=out_specs,
                  check_rep=False),
        keep_unused=True)
    # Output placeholder buffers (the program writes every output element, so
    # no donation/zeroing is needed) — uploaded once and reused every call.
    resident_zeros = [
        jax.device_put(np.zeros((N_CORES * s[0], *s[1:]), d))
        for s, d in zero_shapes]

    def run(per_core_inputs):
        """per_core_inputs: dict name -> global (N_CORES*dim0, ...) array."""
        concat_in = []
        for nm in in_names:
            v = per_core_inputs[nm]
            if isinstance(v, (list, tuple)):
                v = np.concatenate([np.asarray(a) for a in v], axis=0)
            concat_in.append(v)
        out_arrs = sharded(*concat_in, *resident_zeros)
        return {
            nm: np.asarray(out_arrs[i]).reshape(N_CORES, *out_avals[i].shape)
            for i, nm in enumerate(out_names)}
    return run


_NC_CACHE = {}


def kernel(x, w0, w1, w2):
    if "nc" not in _NC_CACHE:
        _NC_CACHE["nc"] = _build_program()
        _NC_CACHE["run"] = _make_runner(_NC_CACHE["nc"])
    run = _NC_CACHE["run"]
    s0, s1, s2 = _host_weights(w0, w1, w2)
    x = np.ascontiguousarray(np.asarray(x, np.float32).reshape(64, H, W))
    res = run({
        "x": x,
        "s0": np.concatenate([s0] * N_CORES, axis=0),
        "s1": np.concatenate([s1] * N_CORES, axis=0),
        "s2": np.concatenate([s2] * N_CORES, axis=0),
    })
    return res["out"].astype(np.float32).reshape(64, 1, H, W)


# revision 17
# speedup vs baseline: 30.8358x; 2.0396x over previous
"""Binary 3-layer CNN (sign activations + sign weights) on 8 NeuronCores.

Strategy: pure data parallel — 64 images -> 8 cores x 8 images.
Per core: 2 batches of 4 images; SBUF partition layout [128 = (4 img, 32 ch), pix].
Each 3x3 conv is computed as PSUM-accumulating fp8 DoubleRow matmuls (K=256
contracted per pass at 2x the bf16 rate). The two DoubleRow k-tiles hold:
  - stage A/B: a pair of dy-taps (dy0&dy2, or dy1&zero) — row-shifted windows
    whose APs are disjoint with strictly descending strides (the PE ifmap AP
    generator rejects overlapping/ascending k-tile strides);
  - stage C: the two image-batches (plane-strided, disjoint).
All matmul operands are exactly +-1/0 -> fp8e4 with fp32 PSUM accumulation is
numerically exact. sign() is applied during PSUM->SBUF eviction (sums are
integer-valued, so clip(x,-1,1) == sign(x)). conv0's output is handed to
conv1 entirely in SBUF (strips overlap by 4 rows so each conv1 strip reads a
single conv0 tile); conv1's output is staged through DRAM in a zero-padded
fp8 layout for the final conv, which emits f16 (exact for |sum|<=288).
PSUM is used as [128, 8, 256] 4-bank tiles so each eviction moves 8 rows in
one instruction; evictions are split ACT (conv0) / DVE (conv1, conv2) to
balance the engines under the PE roofline.

Dispatch: the jitted shard_map callable is built ONCE and cached, so warm
kernel() calls skip re-lowering/re-compile and only pay transfer + execute.
"""

import numpy as np
import ml_dtypes

import jax
import concourse.bass as bass
import concourse.mybir as mybir
import concourse.tile as tile
from concourse import bacc
from concourse.bass_utils import run_bass_kernel_spmd  # noqa: F401 (debug path)
from concourse.bass2jax import (
    _bass_exec_p, partition_id_tensor, install_neuronx_cc_hook)
from jax.experimental.shard_map import shard_map
from jax.sharding import Mesh, PartitionSpec

F32 = mybir.dt.float32
F16 = mybir.dt.float16
FP8 = mybir.dt.float8e4
E4 = ml_dtypes.float8_e4m3
AF = mybir.ActivationFunctionType
DR = mybir.MatmulPerfMode.DoubleRow
ALU = mybir.AluOpType

N_CORES = 8
IMG_PER_CORE = 8
B = 4          # images per partition-batch
H = W = 256
WP = 258       # padded width (1 col pad each side)
HPX = 264      # xs rows: pad row 0, data 1..256, zero rows 257..263
HPH = 258      # h1 rows: pad row 0, data 1..256, pad row 257
R = 64         # strip rows (stages A/B)
RC = 16        # strip rows (stage C)
NB = IMG_PER_CORE // B  # batches per core
TW = 256       # stage-A input tile row width (dx pre-shifted)


def _build_program(stages=('0', 'A', 'B', 'C')):
    nc = bacc.Bacc("TRN2", target_bir_lowering=False, debug=False)

    x_in = nc.dram_tensor("x", [IMG_PER_CORE, H, W], F32, kind="ExternalInput")
    s0_in = nc.dram_tensor("s0", [12, 2, 2, 128], FP8, kind="ExternalInput")
    s1_in = nc.dram_tensor("s1", [128, 6, 2, 128], FP8, kind="ExternalInput")
    # stage-C M padded 8 -> 16: DoubleRow ldweights requires the k-tile
    # stride to be a multiple of 16 bytes (s3_lw dual-fp8 restriction).
    s2_in = nc.dram_tensor("s2", [128, 9, 2, 16], FP8, kind="ExternalInput")
    out_d = nc.dram_tensor("out", [IMG_PER_CORE, H, W], F16, kind="ExternalOutput")

    xs_d = nc.dram_tensor("xs", [IMG_PER_CORE, HPX, WP], FP8)
    h1_d = nc.dram_tensor("h1", [NB, 128, HPH, WP], FP8)

    do = lambda st: st in stages

    with tile.TileContext(nc) as tc:
        with (
            tc.tile_pool(name="const", bufs=1) as cpool,
            tc.tile_pool(name="xprep", bufs=3) as xpool,
            tc.tile_pool(name="ain", bufs=2) as apool,
            tc.tile_pool(name="h0buf", bufs=3) as hpool,
            tc.tile_pool(name="bout", bufs=2) as bopool,
            tc.tile_pool(name="cin", bufs=2) as cpool2,
            tc.tile_pool(name="cout", bufs=2) as copool,
            tc.tile_pool(name="psum", bufs=2, space="PSUM") as pspool,
        ):
            # --- constants: stationary weights + a zero tile ---
            s0t = cpool.tile([12, 2, 2, 128], FP8, tag="s0")
            nc.sync.dma_start(out=s0t[:, :, :, :], in_=s0_in[:, :, :, :])
            s1t = cpool.tile([128, 6, 2, 128], FP8, tag="s1")
            nc.sync.dma_start(out=s1t[:, :, :, :], in_=s1_in[:, :, :, :])
            s2t = cpool.tile([128, 9, 2, 16], FP8, tag="s2")
            nc.sync.dma_start(out=s2t[:, :, :, :], in_=s2_in[:, :, :, :])
            zt = cpool.tile([128, 7 * WP], FP8, tag="zt")
            nc.gpsimd.memset(zt[:, :], 0.0)

            # --- pre-zero DRAM pad rows (SWDGE queue; latency-insensitive,
            # keeps the HWDGE slot free). Col pads are baked into tiles. ---
            for img in range(IMG_PER_CORE):
                nc.gpsimd.dma_start(out=xs_d[img, 0:1, :], in_=zt[0:1, 0:WP])
                nc.gpsimd.dma_start(
                    out=xs_d[img, H + 1:HPX, :],
                    in_=zt[0:1, 0:(HPX - H - 1) * WP])
            for b in range(NB):
                nc.gpsimd.dma_start(out=h1_d[b, :, 0, :], in_=zt[:, 0:WP])
                nc.gpsimd.dma_start(
                    out=h1_d[b, :, H + 1:HPH, :],
                    in_=zt[:, 0:(HPH - H - 1) * WP])

            # --- stage 0: sign(x) -> padded fp8 planes in DRAM (2 imgs/tile) ---
            for i0 in range(0, IMG_PER_CORE if do('0') else 0, 2):
                for rb in range(H // 128):
                    rows = slice(rb * 128, (rb + 1) * 128)
                    xf = xpool.tile([128, 2, W], F32, tag="xf")
                    nc.sync.dma_start(
                        out=xf[:, :, :],
                        in_=x_in[i0:i0 + 2, rows, :].transpose([1, 0, 2]))
                    xp = xpool.tile([128, 2, WP], FP8, tag="xp")
                    nc.scalar.activation(xp[:, :, 1:W + 1], xf[:, :, :], AF.Sign)
                    nc.vector.memset(xp[:, :, 0:1], 0.0)
                    nc.vector.memset(xp[:, :, WP - 1:WP], 0.0)
                    nc.scalar.dma_start(
                        out=xs_d[i0:i0 + 2, rb * 128 + 1:(rb + 1) * 128 + 1,
                                 :].transpose([1, 0, 2]),
                        in_=xp[:, :, :])

            def emit_a(b, s, out_tile):
                # ---- stage A: conv0 (1 -> 32ch). Input partitions =
                # (dx, img); tile row w = xs[img, s*R-1+w, dx:dx+256].
                # Output tile hA row a = h0 padded row v = s*R+a; strips
                # overlap so stage B reads exactly one hA tile.
                # DoubleRow k-tiles = dy pairs: j0=(dy0,dy2), j1=(dy1,0).
                # Generator: yields once per 2-pair psum group so the
                # driver can interleave A and B on the in-order PE queue.
                hA = hpool.tile([128, R + 4, WP], FP8, tag="h0")
                out_tile.append(hA)
                if not do('A'):
                    return
                xt = apool.tile([12, R + 8, TW], FP8, tag="xt")
                lo = s * R - 1  # xs row held at tile row w=0
                for dx in range(3):
                    if lo < 0:
                        nc.sync.dma_start(
                            out=xt[4 * dx:4 * dx + 4, 1:R + 8, :],
                            in_=xs_d[b * B:(b + 1) * B, 0:lo + R + 8,
                                     dx:dx + TW])
                    else:
                        nc.sync.dma_start(
                            out=xt[4 * dx:4 * dx + 4, :, :],
                            in_=xs_d[b * B:(b + 1) * B, lo:lo + R + 8,
                                     dx:dx + TW])
                if lo < 0:
                    nc.vector.memset(xt[:, 0:1, :], 0.0)
                nc.vector.memset(hA[:, :, 0:1], 0.0)
                nc.vector.memset(hA[:, :, WP - 1:WP], 0.0)
                xbase = xt[:, 0:1, 0:1]
                xpd = list(xbase.ap[0])
                # pairs a0: rows (v=s*R+a0, +1); drop pairs that are
                # entirely padding (v > 256), memset them instead.
                npair = 33 if s == H // R - 1 else 34
                for g0 in range(0, npair, 2):
                    nq = min(2, npair - g0)
                    ps = pspool.tile([128, 4, 256], F32, tag="psa", name="psa")
                    for j in range(2):
                        for q in range(nq):
                            a0 = 2 * (g0 + q)
                            rhs = bass.AP(
                                xbase.tensor,
                                xbase.offset + (a0 + j) * TW,
                                [xpd, [2 * TW, 2], [TW, 2], [1, 256]])
                            nc.tensor.matmul(
                                ps[:, 2 * q:2 * q + 2, :],
                                s0t[:, j, :, :], rhs,
                                start=(j == 0), stop=(j == 1),
                                perf_mode=DR)
                    nc.scalar.activation(
                        hA[:, 2 * g0:2 * g0 + 2 * nq, 1:W + 1],
                        ps[:, 0:2 * nq, :], AF.Sign)
                    yield
                # zero the rows that are h0 padding, not data:
                if s == 0:
                    nc.vector.memset(hA[:, 0:1, 1:W + 1], 0.0)
                if s == H // R - 1:
                    nc.vector.memset(hA[:, R + 1:R + 4, 1:W + 1], 0.0)

            def emit_b(b, s, hA):
                # ---- stage B: conv1 (32 -> 32ch), K=128 block-diag,
                # reads hA from SBUF. DoubleRow k-tiles = dy pairs per
                # dx: j<3: (dy0,dx=j)&(dy2,dx=j); j>=3: (dy1,dx=j-3)&0.
                if not do('B'):
                    return
                ht = bopool.tile([128, R, WP], FP8, tag="b_out")
                nc.vector.memset(ht[:, :, 0:1], 0.0)
                nc.vector.memset(ht[:, :, WP - 1:WP], 0.0)
                hbase = hA[:, 0:1, 0:1]
                hpd = list(hbase.ap[0])
                for r0 in range(0, R, 4):
                    ps = pspool.tile([128, 4, 256], F32, tag="psb", name="psb")
                    for j in range(6):
                        dy0 = 0 if j < 3 else 1
                        dx = j % 3
                        for q in range(2):
                            r = r0 + 2 * q
                            rhs = bass.AP(
                                hbase.tensor,
                                hbase.offset + (r + dy0) * WP + dx,
                                [hpd, [2 * WP, 2], [WP, 2], [1, 256]])
                            nc.tensor.matmul(
                                ps[:, 2 * q:2 * q + 2, :],
                                s1t[:, j, :, :], rhs,
                                start=(j == 0), stop=(j == 5),
                                perf_mode=DR)
                    nc.vector.tensor_scalar(
                        ht[:, r0:r0 + 4, 1:W + 1], ps[:, :, :],
                        -1.0, 1.0, op0=ALU.max, op1=ALU.min)
                    yield
                nc.scalar.dma_start(
                    out=h1_d[b, :, s * R + 1:s * R + R + 1, :],
                    in_=ht[:, :, :])

            # Group-level software pipelining: the PE queue is strictly
            # in-order, so B(u) groups are interleaved 1:1 with A(u+1)
            # groups — eviction lag in either stream is absorbed while the
            # PE runs the other stream's matmuls instead of stalling.
            units = ([(b, s) for b in range(NB) for s in range(H // R)]
                     if do('A') or do('B') else [])
            holders = {}

            def start_a(i):
                holders[i] = []
                g = emit_a(*units[i], holders[i])
                return g

            if units:
                g = start_a(0)
                for _ in g:
                    pass
            for i, (b, s) in enumerate(units):
                bgen = emit_b(b, s, holders[i][0])
                agen = start_a(i + 1) if i + 1 < len(units) else iter(())
                done_b = done_a = False
                while not (done_b and done_a):
                    if not done_b:
                        try:
                            next(bgen)
                        except StopIteration:
                            done_b = True
                    if not done_a:
                        try:
                            next(agen)
                        except StopIteration:
                            done_a = True

            # ---- stage C: conv2 (32 -> 1ch), both batches at once:
            # DoubleRow k-tiles = the 2 image-batches (K=256), M=16 (8 imgs
            # + 8 zero-pad columns for the 16B lw k-tile stride rule) ----
            for s in range(H // RC if do('C') else 0):
                hin = cpool2.tile([128, 2, RC + 2, WP], FP8, tag="c_in")
                for b in range(NB):
                    nc.sync.dma_start(
                        out=hin[:, b, :, :],
                        in_=h1_d[b, :, s * RC:s * RC + RC + 2, :])
                ot = copool.tile([8, RC, W], F16, tag="c_out")
                plane = (RC + 2) * WP
                cbase = hin[:, 0:1, 0:1, 0:1]
                cpd = list(cbase.ap[0])
                for r0 in range(0, RC, 4):
                    ps = pspool.tile([128, 4, 256], F32, tag="psa", name="psa")
                    for t in range(9):
                        dy, dx = t // 3, t % 3
                        for q in range(2):
                            r = r0 + 2 * q
                            rhs = bass.AP(
                                cbase.tensor,
                                cbase.offset + (r + dy) * WP + dx,
                                [cpd, [plane, 2], [WP, 2], [1, 256]])
                            nc.tensor.matmul(
                                ps[0:16, 2 * q:2 * q + 2, :],
                                s2t[:, t, :, :], rhs,
                                start=(t == 0), stop=(t == 8),
                                perf_mode=DR)
                    if (r0 // 4) % 2 == 0:
                        nc.vector.tensor_copy(
                            ot[:, r0:r0 + 4, :], ps[0:8, :, :])
                    else:
                        nc.scalar.activation(
                            ot[:, r0:r0 + 4, :], ps[0:8, :, :], AF.Copy)
                nc.scalar.dma_start(
                    out=out_d[0:IMG_PER_CORE, s * RC:(s + 1) * RC, :],
                    in_=ot[:, :, :])
    nc.compile()
    return nc


def _host_weights(w0, w1, w2):
    """Pack sign(w) into fp8 DoubleRow stationary layouts."""
    sg = lambda w: np.sign(np.asarray(w, np.float32))
    w0s, w1s, w2s = sg(w0), sg(w1), sg(w2)  # [32,1,3,3],[32,32,3,3],[1,32,3,3]
    s0 = np.zeros((12, 2, 2, 128), np.float32)
    s1 = np.zeros((128, 6, 2, 128), np.float32)
    s2 = np.zeros((128, 9, 2, 16), np.float32)
    for g in range(B):
        ms = slice(g * 32, (g + 1) * 32)
        for dx in range(3):
            p = dx * 4 + g
            # lhsT[k, kt, m]: out[m] += sum_k sum_kt lhsT[k,kt,m]*rhs[k,kt]
            s0[p, 0, 0, ms] = w0s[:, 0, 0, dx]
            s0[p, 0, 1, ms] = w0s[:, 0, 2, dx]
            s0[p, 1, 0, ms] = w0s[:, 0, 1, dx]
        for j in range(6):
            dx = j % 3
            if j < 3:
                s1[ms, j, 0, ms] = w1s[:, :, 0, dx].T  # [ci, co]
                s1[ms, j, 1, ms] = w1s[:, :, 2, dx].T
            else:
                s1[ms, j, 0, ms] = w1s[:, :, 1, dx].T
        for t in range(9):
            dy, dx = t // 3, t % 3
            for kt in range(NB):
                s2[ms, t, kt, kt * 4 + g] = w2s[0, :, dy, dx]
    return s0.astype(E4), s1.astype(E4), s2.astype(E4)


def _make_runner(nc):
    """Build the jitted shard_map dispatcher ONCE (cached across calls)."""
    install_neuronx_cc_hook()
    partition_name = nc.partition_id_tensor.name if nc.partition_id_tensor else None
    in_names, out_names, out_avals, zero_shapes = [], [], [], []
    for alloc in nc.m.functions[0].allocations:
        if not isinstance(alloc, mybir.MemoryLocationSet):
            continue
        name = alloc.memorylocations[0].name
        if alloc.kind == "ExternalInput":
            if name != partition_name:
                in_names.append(name)
        elif alloc.kind == "ExternalOutput":
            out_names.append(name)
            shape = tuple(alloc.tensor_shape)
            dtype = mybir.dt.np(alloc.dtype)
            out_avals.append(jax.core.ShapedArray(shape, dtype))
            zero_shapes.append((shape, dtype))
    n_params = len(in_names)
    all_in_names = list(in_names) + list(out_names)
    if partition_name is not None:
        all_in_names.append(partition_name)

    def _body(*args):
        operands = list(args)
        if partition_name is not None:
            operands.append(partition_id_tensor())
        outs = _bass_exec_p.bind(
            *operands,
            out_avals=tuple(out_avals),
            in_names=tuple(all_in_names),
            out_names=tuple(out_names),
            lowering_input_output_aliases=(),
            sim_require_finite=True,
            sim_require_nnan=True,
            nc=nc,
        )
        return tuple(outs)

    devices = jax.devices()[:N_CORES]
    mesh = Mesh(np.asarray(devices), ("core",))
    n_outs = len(out_names)
    in_specs = (PartitionSpec("core"),) * (n_params + n_outs)
    out_specs = (PartitionSpec("core"),) * n_outs
    sharded = jax.jit(
        shard_map(_body, mesh=mesh, in_specs=in_specs, out_specs=out_specs,
                  check_rep=False),
        keep_unused=True)
    # Output placeholder buffers (the program writes every output element, so
    # no donation/zeroing is needed) — uploaded once and reused every call.
    resident_zeros = [
        jax.device_put(np.zeros((N_CORES * s[0], *s[1:]), d))
        for s, d in zero_shapes]

    def run(per_core_inputs):
        """per_core_inputs: dict name -> global (N_CORES*dim0, ...) array."""
        concat_in = []
        for nm in in_names:
            v = per_core_inputs[nm]
            if isinstance(v, (list, tuple)):
                v = np.concatenate([np.asarray(a) for a in v], axis=0)
            concat_in.append(v)
        out_arrs = sharded(*concat_in, *resident_zeros)
        return {
            nm: np.asarray(out_arrs[i]).reshape(N_CORES, *out_avals[i].shape)
            for i, nm in enumerate(out_names)}
    return run


_NC_CACHE = {}


def kernel(x, w0, w1, w2):
    if "nc" not in _NC_CACHE:
        _NC_CACHE["nc"] = _build_program()
        _NC_CACHE["run"] = _make_runner(_NC_CACHE["nc"])
    run = _NC_CACHE["run"]
    s0, s1, s2 = _host_weights(w0, w1, w2)
    x = np.ascontiguousarray(np.asarray(x, np.float32).reshape(64, H, W))
    res = run({
        "x": x,
        "s0": np.concatenate([s0] * N_CORES, axis=0),
        "s1": np.concatenate([s1] * N_CORES, axis=0),
        "s2": np.concatenate([s2] * N_CORES, axis=0),
    })
    return res["out"].astype(np.float32).reshape(64, 1, H, W)
